# revision 31
# baseline (speedup 1.0000x reference)
"""3-layer GAT (PyG GATConv, heads=4, concat=False) on 8 Trainium2 NeuronCores.

Strategy (per core, dst-sharded), v2 (bf16):
  - Nodes split into 8 contiguous dst ranges of NV=N/8; edges partitioned by dst
    and sorted; each core processes its dst shard's edges and emits its h rows.
  - Per layer, each core redundantly computes the full node-feature table
    xh_aug = x @ Waug.T  ([N, 260] bf16: 256 per-head features + 4 "as"
    attention columns host-folded into the weight matrix) into its private
    DRAM (rows padded to 384 bf16 = 768B so dma_gather rows are 256B-aligned),
    split into two 25k-row halves so gather indices fit int16.
  - Per 128-dst-node chunk, per-edge rows are fetched with SWDGE dma_gather
    (768B bf16), one multi-packet call per lo/hi section; pad slots fetch row 0
    (skipping them starves some SDMA engines of descriptors and deadlocks the
    completion semaphore). Per-edge ad_dst is NOT gathered: the host uploads
    S^T (the transposed one-hot dst-selection matrix, bf16) and ad values live
    in SBUF
    ([128, 49, 4] per layer: layer-0 from host, else from the epilogue); a tiny
    matmul per subtile (lhsT = S^T slice, rhs = ad_chunk) broadcasts ad to edge
    slots.  e = exp(leakyrelu0.2(as_src + ad_dst)) is computed in f32 on
    DVE/ACT and written (bf16) into the gather rows' as-columns, so the one-hot
    aggregation matmuls carry the softmax denominator as 4 extra columns.
    Softmax max-subtraction is skipped (logits are O(10); exp is safe in f32).
  - Epilogue: divide by denominator, mean heads, +bias +residual, leaky;
    h chunk is PE-transposed (bf16) for the next layer's matmuls, and the next
    layer's ad values are computed immediately (tiny matmul) into SBUF.
  - Between layers: AllGather of each core's h^T block (the only collective).

All cores run one SPMD program: per-chunk subtile counts are maxed over cores;
pad slots fetch table row 0 and carry S/S^T zero columns, so they contribute
exactly zero.  Phase-1 writes interleave nodes so each partition emits 12
consecutive table rows, turning the table store into 9.2KB-per-partition
descriptors instead of per-row 768B ones.
"""
import numpy as np

N = 50000
E0 = 800000
NCORES = 8
NV = N // NCORES          # 6250 dst nodes per core
P = 128
NCHUNK = (NV + P - 1) // P  # 49
HALF = N // 2             # table split for int16 gather indices
H, F = 4, 64
C = H * F                 # 256
CA = C + H                # 260 live columns (feat + as)
ROW = 384                 # bf16 elems per table row (768B, 256B-aligned)
IN = 128
HID = 64

_cache = {}


def _wrap_idx(flat):
    """[G] int -> dma_gather wrapped layout [128, G//16] int16."""
    G = len(flat)
    assert G % 16 == 0
    w = np.asarray(flat, np.int16).reshape(G // 16, 16).T  # [16, G//16]
    return np.tile(w, (8, 1))                              # [128, G//16]


def _fold_attn(W, a):
    # as_n[h] = sum_f a[0,h,f] * (W x)_{h*F+f} -> fold into weight rows: [H, in]
    return np.einsum("hf,hfi->hi", a[0], W.reshape(H, F, W.shape[1]))


def _preprocess(x, edge_index, weights):
    """Host-side edge partitioning + per-core input maps + shared meta."""
    (W1, as1, ad1, b1, W2, as2, ad2, b2, W3, as3, ad3, b3,
     rw1, rb1, rw2, rb2) = weights

    loops = np.arange(N, dtype=np.int64)
    src = np.concatenate([np.asarray(edge_index[0]), loops])
    dst = np.concatenate([np.asarray(edge_index[1]), loops])
    order = np.argsort(dst, kind="stable")
    ssrc = src[order].astype(np.int64)
    sdst = dst[order].astype(np.int64)

    # chunk boundaries: (core, chunk) -> edge ranges; edges split lo/hi by src,
    # each section sorted by src (ascending gather addresses).
    edges_lo = [[None] * NCHUNK for _ in range(NCORES)]
    edges_hi = [[None] * NCHUNK for _ in range(NCORES)]
    n_lo = np.zeros((NCORES, NCHUNK), np.int64)
    n_hi = np.zeros((NCORES, NCHUNK), np.int64)
    for c in range(NCORES):
        for k in range(NCHUNK):
            gbase = c * NV + k * P
            gend = min(gbase + P, (c + 1) * NV)
            a = np.searchsorted(sdst, gbase)
            b = np.searchsorted(sdst, gend)
            es, ed = ssrc[a:b], sdst[a:b]
            lo = es < HALF
            for part, m in ((0, lo), (1, ~lo)):
                pe, pd = es[m], ed[m]
                o = np.argsort(pe, kind="stable")
                pe, pd = pe[o], pd[o]
                if part == 0:
                    edges_lo[c][k] = (pe, pd)
                    n_lo[c][k] = len(pe)
                else:
                    edges_hi[c][k] = (pe, pd)
                    n_hi[c][k] = len(pe)

    NLO = ((n_lo.max(0) + P - 1) // P).astype(int)   # shared subtile counts
    NHI = ((n_hi.max(0) + P - 1) // P).astype(int)
    NSUB = NLO + NHI
    TOT = int(NSUB.sum())
    sub_off = np.zeros(NCHUNK, int)                   # subtile offset per chunk
    sub_off[1:] = np.cumsum(NSUB)[:-1]

    import ml_dtypes
    bf16 = ml_dtypes.bfloat16

    # folded weights (bf16)
    def waugT(W, a_s):
        # [in, 260]: cols 0:256 = W.T ; 256:260 = as-fold
        out = np.zeros((W.shape[1], CA), np.float32)
        out[:, :C] = W.T
        out[:, C:] = _fold_attn(W, a_s).T
        return out.astype(bf16)

    w1t = waugT(np.asarray(W1, np.float32), np.asarray(as1, np.float32))
    w2t = waugT(np.asarray(W2, np.float32), np.asarray(as2, np.float32))
    w3t = waugT(np.asarray(W3, np.float32), np.asarray(as3, np.float32))
    adf1 = _fold_attn(np.asarray(W1, np.float32), np.asarray(ad1, np.float32)).T  # [in,4]
    adf2 = _fold_attn(np.asarray(W2, np.float32), np.asarray(ad2, np.float32)).T  # [64,4]
    adf3 = _fold_attn(np.asarray(W3, np.float32), np.asarray(ad3, np.float32)).T
    bias = np.stack([np.asarray(b1), np.asarray(b2), np.asarray(b3)]).astype(np.float32)
    bias_rep = np.tile(bias[None], (P, 1, 1))         # [128, 3, 64]
    iota = np.tile(np.arange(P, dtype=np.float32), (P, 1)).astype(bf16)  # [128,128]

    xf = np.asarray(x, np.float32)
    xT = np.ascontiguousarray(xf.T).astype(bf16)      # [128, N] bf16
    xres = np.stack([xf @ np.asarray(rw1, np.float32).T + np.asarray(rb1, np.float32),
                     xf @ np.asarray(rw2, np.float32).T + np.asarray(rb2, np.float32)])

    in_maps = []
    for c in range(NCORES):
        # pad slots gather row 0 (a skipped all-negative call would issue no
        # descriptors on some SDMA engines and deadlock the completion sem)
        gidx = np.zeros(TOT * P, np.int64)
        onehot = np.full(TOT * P, 255.0, np.float32)
        for k in range(NCHUNK):
            base = sub_off[k] * P
            gbase = c * NV + k * P
            for part, (es, ed), nsub_off in (
                (0, edges_lo[c][k], 0),
                (1, edges_hi[c][k], NLO[k] * P),
            ):
                o = base + nsub_off
                n = len(es)
                tv = es if part == 0 else es - HALF
                gidx[o:o + n] = tv
                onehot[o:o + n] = (ed - gbase).astype(np.float32)
        # S^T: [128, TOT*128] bf16; column (J*128+s) is one-hot at dst-local.
        oh2 = onehot.reshape(TOT, P)                   # [J, s]
        st = np.zeros((P, TOT * P), bf16)
        jj, ss = np.nonzero(oh2 != 255.0)
        st[oh2[jj, ss].astype(np.int64), jj * P + ss] = 1.0
        # ad1 values for this core's nodes, [128, 49, 8] (layer-1 ad is
        # input-only); row (k,p) = node c*NV + k*128 + p.
        ad1_sb = np.zeros((P, NCHUNK, 8), np.float32)
        myx = xf[c * NV:(c + 1) * NV] @ adf1           # [NV, 4]
        pad = np.zeros((NCHUNK * P - NV, H), np.float32)
        ad1_sb[:, :, :H] = np.concatenate([myx, pad]).reshape(NCHUNK, P, H).transpose(1, 0, 2)
        in_maps.append({
            "xT": xT,
            "gidx": _wrap_idx(gidx),
            "onehot": np.ascontiguousarray(
                onehot.reshape(TOT, P).T).astype(bf16),  # [128, TOT]
            "stt": st,
            "iota": iota,
            "w1t": w1t, "w2t": w2t, "w3t": w3t,
            "adf2": adf2.astype(bf16), "adf3": adf3.astype(bf16),
            "bias": bias_rep,
            "xres": xres[:, c * NV:(c + 1) * NV].astype(np.float32),
            "ad1": ad1_sb.astype(bf16),
        })

    meta = dict(NLO=NLO, NHI=NHI, NSUB=NSUB, TOT=TOT, sub_off=sub_off)
    return in_maps, meta


def _build_program(meta):
    import concourse.bass as bass
    import concourse.bacc as bacc
    import concourse.tile as tile
    import concourse.mybir as mybir
    from concourse import library_config
    from concourse.masks import make_identity

    AF = mybir.ActivationFunctionType
    ALU = mybir.AluOpType
    f32 = mybir.dt.float32
    bf16 = mybir.dt.bfloat16
    i16 = mybir.dt.int16

    NLO, NHI, NSUB = meta["NLO"], meta["NHI"], meta["NSUB"]
    TOT, sub_off = meta["TOT"], meta["sub_off"]
    NSUBMAX = int(NSUB.max())

    NQ = 4
    nc = bacc.Bacc("TRN2", num_devices=NCORES, num_swdge_queues=NQ)

    # ---- I/O ----
    t_xT = nc.dram_tensor("xT", [IN, N], bf16, kind="ExternalInput")
    t_gidx = nc.dram_tensor("gidx", [P, TOT * 8], i16, kind="ExternalInput")
    t_oh = nc.dram_tensor("onehot", [P, TOT], bf16, kind="ExternalInput")
    t_stt = nc.dram_tensor("stt", [P, TOT * P], bf16, kind="ExternalInput")
    t_iota = nc.dram_tensor("iota", [P, P], bf16, kind="ExternalInput")
    t_w = [nc.dram_tensor("w1t", [IN, CA], bf16, kind="ExternalInput"),
           nc.dram_tensor("w2t", [HID, CA], bf16, kind="ExternalInput"),
           nc.dram_tensor("w3t", [HID, CA], bf16, kind="ExternalInput")]
    t_adf = [None,
             nc.dram_tensor("adf2", [HID, H], bf16, kind="ExternalInput"),
             nc.dram_tensor("adf3", [HID, H], bf16, kind="ExternalInput")]
    t_bias = nc.dram_tensor("bias", [P, 3, HID], f32, kind="ExternalInput")
    t_xres = nc.dram_tensor("xres", [2, NV, HID], f32, kind="ExternalInput")
    t_ad1 = nc.dram_tensor("ad1", [P, NCHUNK, 8], bf16, kind="ExternalInput")
    GMAX = 8
    t_out = nc.dram_tensor("out", [NV, HID], f32, kind="ExternalOutput")

    with tile.TileContext(nc) as tc:
        import contextlib
        with contextlib.ExitStack() as ctx:
            dram = ctx.enter_context(tc.tile_pool(name="dram", bufs=1, space="DRAM"))
            sb_res = ctx.enter_context(tc.tile_pool(name="res", bufs=1))
            sb_slab = ctx.enter_context(tc.tile_pool(name="slab", bufs=2))
            sb_p1 = ctx.enter_context(tc.tile_pool(name="p1", bufs=3))
            sb_s = ctx.enter_context(tc.tile_pool(name="s", bufs=2))
            sb_sm = ctx.enter_context(tc.tile_pool(name="sm", bufs=2))
            sb_ep = ctx.enter_context(tc.tile_pool(name="ep", bufs=2))
            ps_p1 = ctx.enter_context(tc.tile_pool(name="psp1", bufs=2, space="PSUM"))
            ps_acc = ctx.enter_context(tc.tile_pool(name="psacc", bufs=2, space="PSUM"))
            ps_ad = ctx.enter_context(tc.tile_pool(name="psad", bufs=2, space="PSUM"))
            ps_tr = ctx.enter_context(tc.tile_pool(name="pstr", bufs=1, space="PSUM"))

            nc.gpsimd.load_library(library_config.mlp)

            # DRAM intermediates (per-core private)
            T_lo = dram.tile([HALF + 128, ROW], bf16)
            T_hi = dram.tile([HALF + 128, ROW], bf16)
            hT_mine = [dram.tile([HID, NV], bf16, tag="hTm1", name="hTm1"),
                       dram.tile([HID, NV], bf16, tag="hTm2", name="hTm2")]
            hT_full = [dram.tile([NCORES, HID, NV], bf16, addr_space="Shared", tag="hTf1", name="hTf1"),
                       dram.tile([NCORES, HID, NV], bf16, addr_space="Shared", tag="hTf2", name="hTf2")]

            # ---- resident tiles ----
            gidx = sb_res.tile([P, TOT * 8], i16)
            oh = sb_res.tile([P, TOT], bf16)
            iota = sb_res.tile([P, P], bf16)
            bias = sb_res.tile([P, 3, HID], f32)
            wt = [sb_res.tile([IN, CA], bf16, tag="w1", name="w1"),
                  sb_res.tile([HID, CA], bf16, tag="w2", name="w2"),
                  sb_res.tile([HID, CA], bf16, tag="w3", name="w3")]
            adf = [None,
                   sb_res.tile([HID, H], bf16, tag="adf2", name="adf2"),
                   sb_res.tile([HID, H], bf16, tag="adf3", name="adf3")]
            ident = sb_res.tile([P, P], bf16)
            # ad values, ping-pong by layer parity: [128, 2, 49, 8]
            ad_sb = sb_res.tile([P, 2, NCHUNK, 8], bf16, tag="adsb", name="adsb")
            # two manually ping-ponged per-edge feature tiles (memset once so
            # pad slots always hold finite data)
            gbuf = [sb_res.tile([P, NSUBMAX, ROW], bf16, tag="g0", name="g0"),
                    sb_res.tile([P, NSUBMAX, ROW], bf16, tag="g1", name="g1")]
            make_identity(nc, ident[:])
            nc.sync.dma_start(gidx[:], t_gidx[:])
            nc.sync.dma_start(oh[:], t_oh[:])
            nc.sync.dma_start(iota[:], t_iota[:])
            nc.sync.dma_start(bias[:], t_bias[:])
            nc.sync.dma_start(ad_sb[:, 0], t_ad1[:])
            for i in range(3):
                nc.sync.dma_start(wt[i][:], t_w[i][:])
            for i in (1, 2):
                nc.sync.dma_start(adf[i][:], t_adf[i][:])

            # =========================================================
            # per-layer
            # =========================================================
            for layer in range(3):
                kin = IN if layer == 0 else HID

                # ---- phase 1: full-table xh_aug = in @ WaugT ----
                # Full 1536-row slabs interleave nodes so partition p emits 12
                # consecutive table rows -> one DMA of 128 x 9.2KB descriptors
                # instead of per-row 768B writes.  Remainders use _p1_tile.
                SLABW = 12 * P
                for half, T_tab in ((0, T_lo), (1, T_hi)):
                    if layer == 0:
                        srcs = [(t_xT, half * HALF, HALF, 0)]
                    else:
                        hfull = hT_full[layer - 1]
                        srcs = [(hfull[half * 4 + bb], 0, NV, bb * NV)
                                for bb in range(4)]
                    for srct, scol, slen, rbase in srcs:
                        for s0 in range(0, slen, SLABW):
                            w = min(SLABW, slen - s0)
                            slab = sb_slab.tile([kin, SLABW], bf16, tag="slab")
                            if layer == 0:
                                nc.sync.dma_start(
                                    slab[:, :w], srct[:, scol + s0:scol + s0 + w])
                            else:
                                nc.sync.dma_start(
                                    slab[:, :w], srct[:, s0:s0 + w])
                            if w == SLABW:
                                osb = sb_p1.tile([P, 12, ROW], bf16, tag="oslab")
                                sv = slab.rearrange("i (q j) -> i q j", j=12)
                                for j in range(12):
                                    ps = ps_p1.tile([P, CA], f32, space="PSUM",
                                                    tag="p1ps")
                                    nc.tensor.matmul(out=ps[:], lhsT=sv[:, :, j],
                                                     rhs=wt[layer][:],
                                                     start=True, stop=True)
                                    nc.scalar.activation(osb[:, j, 0:CA], ps[:],
                                                         AF.Copy)
                                dst = T_tab[rbase + s0:rbase + s0 + SLABW, :]
                                nc.sync.dma_start(
                                    dst.rearrange("(p j) c -> p j c", p=P),
                                    osb[:])
                            else:
                                for o in range(0, w, P):
                                    m = min(P, w - o)
                                    _p1_tile(nc, tc, slab[:, o:o + m], wt[layer],
                                             T_tab, rbase + s0 + o, m, kin,
                                             sb_p1, ps_p1, mybir)

                # ---- phase 2: per dst-chunk edge processing ----
                for k in range(NCHUNK):
                    m = min(P, NV - k * P)
                    nlo, nhi, nsub = int(NLO[k]), int(NHI[k]), int(NSUB[k])
                    so = int(sub_off[k])

                    g = gbuf[k % 2]
                    for s0, s1, tab in ((0, nlo, T_lo), (nlo, nsub, T_hi)):
                        nn = (s1 - s0) * P
                        nc.gpsimd.dma_gather(
                            g[:, s0:s1, :], tab[:],
                            gidx[:, (so + s0) * 8:(so + s1) * 8],
                            nn, nn, ROW,
                            single_packet=(nn <= 1024))

                    # S^T slice for this chunk
                    stt = sb_s.tile([P, NSUBMAX, P], bf16, tag="stt")
                    nc.sync.dma_start(stt[:, 0:nsub, :],
                                      t_stt[:, so * P:(so + nsub) * P])
                    # ad_dst per edge slot: adp[:, j, :] = S^T_j.T @ ad_chunk
                    adp = ps_ad.tile([P, NSUBMAX, H], f32, space="PSUM", tag="adp")
                    for j in range(nsub):
                        nc.tensor.matmul(out=adp[:, j, :], lhsT=stt[:, j, :],
                                         rhs=ad_sb[:, layer % 2, k, 0:H],
                                         start=True, stop=True)

                    # e = exp(prelu0.2(as + ad)), f32 logits
                    lgt = sb_sm.tile([P, NSUBMAX, H], f32, tag="lgt")
                    nc.vector.tensor_tensor(
                        out=lgt[:, 0:nsub, :], in0=g[:, 0:nsub, C:CA],
                        in1=adp[:, 0:nsub, :], op=ALU.add)
                    lk = sb_sm.tile([P, NSUBMAX, H], f32, tag="lk")
                    nc.vector.scalar_tensor_tensor(
                        out=lk[:, 0:nsub, :], in0=lgt[:, 0:nsub, :], scalar=0.2,
                        in1=lgt[:, 0:nsub, :], op0=ALU.mult, op1=ALU.max)
                    # e -> g as-columns (bf16, denominator rides the matmul)
                    nc.scalar.activation(g[:, 0:nsub, C:CA], lk[:, 0:nsub, :],
                                         AF.Exp)
                    # weight messages in place: g[:, :, 0:C] *= e (bcast over F)
                    nc.vector.tensor_tensor(
                        out=g[:, 0:nsub, 0:C].rearrange(
                            "p ns (h f) -> p ns h f", h=H),
                        in0=g[:, 0:nsub, 0:C].rearrange(
                            "p ns (h f) -> p ns h f", h=H),
                        in1=g[:, 0:nsub, C:CA].unsqueeze(-1).to_broadcast(
                            [P, nsub, H, F]),
                        op=ALU.mult)
                    # one-hot selection (slot-major)
                    S = sb_s.tile([P, NSUBMAX, P], bf16, tag="S")
                    nc.vector.tensor_tensor(
                        out=S[:, 0:nsub, :],
                        in0=oh[:, so:so + nsub].unsqueeze(-1).to_broadcast(
                            [P, nsub, P]),
                        in1=iota[:].unsqueeze(1).to_broadcast([P, nsub, P]),
                        op=ALU.is_equal)
                    # aggregate
                    acc = ps_acc.tile([P, CA], f32, space="PSUM", tag="acc")
                    for j in range(nsub):
                        nc.tensor.matmul(out=acc[:], lhsT=S[:, j, :],
                                         rhs=g[:, j, 0:CA],
                                         start=(j == 0), stop=(j == nsub - 1))

                    # ---- epilogue ----
                    rs = sb_ep.tile([P, H], f32, tag="rs")
                    nc.vector.reciprocal(rs[:], acc[:, C:CA])
                    o = sb_ep.tile([P, H, F], f32, tag="o")
                    nc.vector.tensor_tensor(
                        out=o[:],
                        in0=acc[:, 0:C].rearrange("p (h f) -> p h f", h=H),
                        in1=rs[:].unsqueeze(-1).to_broadcast([P, H, F]),
                        op=ALU.mult)
                    o2 = sb_ep.tile([P, 2, F], f32, tag="o2")
                    nc.vector.tensor_tensor(out=o2[:, 0, :], in0=o[:, 0, :],
                                            in1=o[:, 1, :], op=ALU.add)
                    nc.vector.tensor_tensor(out=o2[:, 1, :], in0=o[:, 2, :],
                                            in1=o[:, 3, :], op=ALU.add)
                    hsum = sb_ep.tile([P, F], f32, tag="hsum")
                    nc.vector.tensor_tensor(out=hsum[:], in0=o2[:, 0, :],
                                            in1=o2[:, 1, :], op=ALU.add)
                    # mean + bias
                    hb = sb_ep.tile([P, F], f32, tag="hb")
                    nc.vector.scalar_tensor_tensor(
                        out=hb[:], in0=hsum[:], scalar=0.25, op0=ALU.mult,
                        in1=bias[:, layer, :], op1=ALU.add)
                    # + residual (+ leaky for layers 0,1)
                    res = sb_ep.tile([P, F], f32, tag="res")
                    ri = 0 if layer < 2 else 1
                    nc.sync.dma_start(res[:m], t_xres[ri, k * P:k * P + m, :])
                    hf = sb_ep.tile([P, F], f32, tag="hf")
                    nc.vector.tensor_tensor(out=hf[:m], in0=hb[:m],
                                            in1=res[:m], op=ALU.add)
                    if layer < 2:
                        ho = sb_ep.tile([P, F], bf16, tag="ho")
                        if m < P:
                            nc.vector.memset(ho[:], 0)
                        nc.vector.scalar_tensor_tensor(
                            out=ho[:m], in0=hf[:m], scalar=0.01,
                            in1=hf[:m], op0=ALU.mult, op1=ALU.max)
                        # transpose (bf16) for next layer + store
                        trp = ps_tr.tile([HID, P], bf16, space="PSUM", tag="trp")
                        nc.tensor.transpose(out=trp[:], in_=ho[:, 0:HID],
                                            identity=ident[:])
                        hTt = sb_ep.tile([HID, P], bf16, tag="hTt")
                        nc.scalar.activation(hTt[:], trp[:], AF.Copy)
                        nc.sync.dma_start(hT_mine[layer][:, k * P:k * P + m],
                                          hTt[:, 0:m])
                        # next-layer ad for my rows -> SBUF (other parity)
                        adn = ps_tr.tile([P, H], f32, space="PSUM", tag="adn")
                        nc.tensor.matmul(out=adn[:], lhsT=hTt[:],
                                         rhs=adf[layer + 1][:],
                                         start=True, stop=True)
                        nc.scalar.activation(ad_sb[:, (layer + 1) % 2, k, 0:H],
                                             adn[:], AF.Copy)
                    else:
                        nc.sync.dma_start(t_out[k * P:k * P + m, :], hf[:m])

                # ---- allgather h^T ----
                if layer < 2:
                    nc.gpsimd.collective_compute(
                        "AllGather", mybir.AluOpType.bypass,
                        replica_groups=[list(range(NCORES))],
                        ins=[hT_mine[layer].opt()],
                        outs=[hT_full[layer].opt()])

    nc.compile()
    return nc


def _p1_tile(nc, tc, lhsT, wt, T_tab, rowbase, m, kin, sb_p1, ps_p1, mybir):
    """One phase-1 tile: rows [rowbase, rowbase+m) of the table."""
    f32 = mybir.dt.float32
    bf16 = mybir.dt.bfloat16
    AF = mybir.ActivationFunctionType
    ps = ps_p1.tile([P, CA], f32, space="PSUM", tag="p1ps")
    nc.tensor.matmul(out=ps[:m if m < P else P, :], lhsT=lhsT[:, 0:m],
                     rhs=wt[:], start=True, stop=True)
    sb = sb_p1.tile([P, ROW], bf16, tag="p1sb")
    nc.scalar.activation(sb[:m, 0:CA], ps[:m, :], AF.Copy)
    nc.sync.dma_start(T_tab[rowbase:rowbase + m, 0:CA], sb[:m, 0:CA])


def kernel(**inputs):
    from concourse.bass_utils import run_bass_kernel_spmd

    x = np.asarray(inputs["x"], np.float32)
    ei = np.asarray(inputs["edge_index"])
    weights = tuple(inputs[k] for k in
                    ("W1", "as1", "ad1", "b1", "W2", "as2", "ad2", "b2",
                     "W3", "as3", "ad3", "b3", "rw1", "rb1", "rw2", "rb2"))
    in_maps, meta = _preprocess(x, ei, weights)

    key = ("prog", tuple(meta["NLO"]), tuple(meta["NHI"]))
    if key not in _cache:
        _cache[key] = _build_program(meta)
    nc = _cache[key]

    res = run_bass_kernel_spmd(nc, in_maps, core_ids=list(range(NCORES)))
    out = np.concatenate([res.results[c]["out"] for c in range(NCORES)], axis=0)
    return out.astype(np.float32)


# revision 34
# speedup vs baseline: 1.0104x; 1.0104x over previous
"""3-layer GAT (PyG GATConv, heads=4, concat=False) on 8 Trainium2 NeuronCores.

Strategy (per core, dst-sharded), v2 (bf16):
  - Nodes split into 8 contiguous dst ranges of NV=N/8; edges partitioned by dst
    and sorted; each core processes its dst shard's edges and emits its h rows.
  - Per layer, each core redundantly computes the full node-feature table
    xh_aug = x @ Waug.T  ([N, 260] bf16: 256 per-head features + 4 "as"
    attention columns host-folded into the weight matrix) into its private
    DRAM (rows padded to 384 bf16 = 768B so dma_gather rows are 256B-aligned),
    split into two 25k-row halves so gather indices fit int16.
  - Per 128-dst-node chunk, per-edge rows are fetched with SWDGE dma_gather
    (768B bf16), one multi-packet call per lo/hi section; pad slots fetch row 0
    (skipping them starves some SDMA engines of descriptors and deadlocks the
    completion semaphore). Per-edge ad_dst is NOT gathered: the host uploads
    S^T (the transposed one-hot dst-selection matrix, bf16) and ad values live
    in SBUF
    ([128, 49, 4] per layer: layer-0 from host, else from the epilogue); a tiny
    matmul per subtile (lhsT = S^T slice, rhs = ad_chunk) broadcasts ad to edge
    slots.  e = exp(leakyrelu0.2(as_src + ad_dst)) is computed in f32 on
    DVE/ACT and written (bf16) into the gather rows' as-columns, so the one-hot
    aggregation matmuls carry the softmax denominator as 4 extra columns.
    Softmax max-subtraction is skipped (logits are O(10); exp is safe in f32).
  - Epilogue: divide by denominator, mean heads, +bias +residual, leaky;
    h chunk is PE-transposed (bf16) for the next layer's matmuls, and the next
    layer's ad values are computed immediately (tiny matmul) into SBUF.
  - Between layers: AllGather of each core's h^T block (the only collective).

All cores run one SPMD program: per-chunk subtile counts are maxed over cores;
pad slots fetch table row 0 and carry S/S^T zero columns, so they contribute
exactly zero.  Phase-1 writes interleave nodes so each partition emits 12
consecutive table rows, turning the table store into 9.2KB-per-partition
descriptors instead of per-row 768B ones.
"""
import numpy as np

N = 50000
E0 = 800000
NCORES = 8
NV = N // NCORES          # 6250 dst nodes per core
P = 128
NCHUNK = (NV + P - 1) // P  # 49
HALF = N // 2             # table split for int16 gather indices
H, F = 4, 64
C = H * F                 # 256
CA = C + H                # 260 live columns (feat + as)
ROW = 384                 # bf16 elems per table row (768B, 256B-aligned)
IN = 128
HID = 64

_cache = {}


def _wrap_idx(flat):
    """[G] int -> dma_gather wrapped layout [128, G//16] int16."""
    G = len(flat)
    assert G % 16 == 0
    w = np.asarray(flat, np.int16).reshape(G // 16, 16).T  # [16, G//16]
    return np.tile(w, (8, 1))                              # [128, G//16]


def _fold_attn(W, a):
    # as_n[h] = sum_f a[0,h,f] * (W x)_{h*F+f} -> fold into weight rows: [H, in]
    return np.einsum("hf,hfi->hi", a[0], W.reshape(H, F, W.shape[1]))


def _preprocess(x, edge_index, weights):
    """Host-side edge partitioning + per-core input maps + shared meta."""
    (W1, as1, ad1, b1, W2, as2, ad2, b2, W3, as3, ad3, b3,
     rw1, rb1, rw2, rb2) = weights

    loops = np.arange(N, dtype=np.int64)
    src = np.concatenate([np.asarray(edge_index[0]), loops])
    dst = np.concatenate([np.asarray(edge_index[1]), loops])
    order = np.argsort(dst, kind="stable")
    ssrc = src[order].astype(np.int64)
    sdst = dst[order].astype(np.int64)

    # chunk boundaries: (core, chunk) -> edge ranges; edges split lo/hi by src,
    # each section sorted by src (ascending gather addresses).
    edges_lo = [[None] * NCHUNK for _ in range(NCORES)]
    edges_hi = [[None] * NCHUNK for _ in range(NCORES)]
    n_lo = np.zeros((NCORES, NCHUNK), np.int64)
    n_hi = np.zeros((NCORES, NCHUNK), np.int64)
    for c in range(NCORES):
        for k in range(NCHUNK):
            gbase = c * NV + k * P
            gend = min(gbase + P, (c + 1) * NV)
            a = np.searchsorted(sdst, gbase)
            b = np.searchsorted(sdst, gend)
            es, ed = ssrc[a:b], sdst[a:b]
            lo = es < HALF
            for part, m in ((0, lo), (1, ~lo)):
                pe, pd = es[m], ed[m]
                o = np.argsort(pe, kind="stable")
                pe, pd = pe[o], pd[o]
                if part == 0:
                    edges_lo[c][k] = (pe, pd)
                    n_lo[c][k] = len(pe)
                else:
                    edges_hi[c][k] = (pe, pd)
                    n_hi[c][k] = len(pe)

    NLO = ((n_lo.max(0) + P - 1) // P).astype(int)   # shared subtile counts
    NHI = ((n_hi.max(0) + P - 1) // P).astype(int)
    NSUB = NLO + NHI
    TOT = int(NSUB.sum())
    sub_off = np.zeros(NCHUNK, int)                   # subtile offset per chunk
    sub_off[1:] = np.cumsum(NSUB)[:-1]

    import ml_dtypes
    bf16 = ml_dtypes.bfloat16

    # folded weights (bf16)
    def waugT(W, a_s):
        # [in, 260]: cols 0:256 = W.T ; 256:260 = as-fold
        out = np.zeros((W.shape[1], CA), np.float32)
        out[:, :C] = W.T
        out[:, C:] = _fold_attn(W, a_s).T
        return out.astype(bf16)

    w1t = waugT(np.asarray(W1, np.float32), np.asarray(as1, np.float32))
    w2t = waugT(np.asarray(W2, np.float32), np.asarray(as2, np.float32))
    w3t = waugT(np.asarray(W3, np.float32), np.asarray(as3, np.float32))
    adf1 = _fold_attn(np.asarray(W1, np.float32), np.asarray(ad1, np.float32)).T  # [in,4]
    adf2 = _fold_attn(np.asarray(W2, np.float32), np.asarray(ad2, np.float32)).T  # [64,4]
    adf3 = _fold_attn(np.asarray(W3, np.float32), np.asarray(ad3, np.float32)).T
    bias = np.stack([np.asarray(b1), np.asarray(b2), np.asarray(b3)]).astype(np.float32)
    bias_rep = np.tile(bias[None], (P, 1, 1))         # [128, 3, 64]
    iota = np.tile(np.arange(P, dtype=np.float32), (P, 1)).astype(bf16)  # [128,128]

    xf = np.asarray(x, np.float32)
    xT = np.ascontiguousarray(xf.T).astype(bf16)      # [128, N] bf16
    xres = np.stack([xf @ np.asarray(rw1, np.float32).T + np.asarray(rb1, np.float32),
                     xf @ np.asarray(rw2, np.float32).T + np.asarray(rb2, np.float32)])

    in_maps = []
    for c in range(NCORES):
        # pad slots gather row 0 (a skipped all-negative call would issue no
        # descriptors on some SDMA engines and deadlock the completion sem)
        gidx = np.zeros(TOT * P, np.int64)
        onehot = np.full(TOT * P, 255.0, np.float32)
        for k in range(NCHUNK):
            base = sub_off[k] * P
            gbase = c * NV + k * P
            for part, (es, ed), nsub_off in (
                (0, edges_lo[c][k], 0),
                (1, edges_hi[c][k], NLO[k] * P),
            ):
                o = base + nsub_off
                n = len(es)
                tv = es if part == 0 else es - HALF
                gidx[o:o + n] = tv
                onehot[o:o + n] = (ed - gbase).astype(np.float32)
        # S^T: [128, TOT*128] bf16; column (J*128+s) is one-hot at dst-local.
        oh2 = onehot.reshape(TOT, P)                   # [J, s]
        st = np.zeros((P, TOT * P), bf16)
        jj, ss = np.nonzero(oh2 != 255.0)
        st[oh2[jj, ss].astype(np.int64), jj * P + ss] = 1.0
        # ad1 values for this core's nodes, [128, 49, 8] (layer-1 ad is
        # input-only); row (k,p) = node c*NV + k*128 + p.
        ad1_sb = np.zeros((P, NCHUNK, 8), np.float32)
        myx = xf[c * NV:(c + 1) * NV] @ adf1           # [NV, 4]
        pad = np.zeros((NCHUNK * P - NV, H), np.float32)
        ad1_sb[:, :, :H] = np.concatenate([myx, pad]).reshape(NCHUNK, P, H).transpose(1, 0, 2)
        in_maps.append({
            "xT": xT,
            "gidx": _wrap_idx(gidx),
            "onehot": np.ascontiguousarray(
                onehot.reshape(TOT, P).T).astype(bf16),  # [128, TOT]
            "stt": st,
            "iota": iota,
            "w1t": w1t, "w2t": w2t, "w3t": w3t,
            "adf2": adf2.astype(bf16), "adf3": adf3.astype(bf16),
            "bias": bias_rep,
            "xres": xres[:, c * NV:(c + 1) * NV].astype(np.float32),
            "ad1": ad1_sb.astype(bf16),
        })

    meta = dict(NLO=NLO, NHI=NHI, NSUB=NSUB, TOT=TOT, sub_off=sub_off)
    return in_maps, meta


def _build_program(meta):
    import concourse.bass as bass
    import concourse.bacc as bacc
    import concourse.tile as tile
    import concourse.mybir as mybir
    from concourse import library_config
    from concourse.masks import make_identity

    AF = mybir.ActivationFunctionType
    ALU = mybir.AluOpType
    f32 = mybir.dt.float32
    bf16 = mybir.dt.bfloat16
    i16 = mybir.dt.int16

    NLO, NHI, NSUB = meta["NLO"], meta["NHI"], meta["NSUB"]
    TOT, sub_off = meta["TOT"], meta["sub_off"]
    NSUBMAX = int(NSUB.max())

    NQ = 4
    nc = bacc.Bacc("TRN2", num_devices=NCORES, num_swdge_queues=NQ)

    # ---- I/O ----
    t_xT = nc.dram_tensor("xT", [IN, N], bf16, kind="ExternalInput")
    t_gidx = nc.dram_tensor("gidx", [P, TOT * 8], i16, kind="ExternalInput")
    t_oh = nc.dram_tensor("onehot", [P, TOT], bf16, kind="ExternalInput")
    t_stt = nc.dram_tensor("stt", [P, TOT * P], bf16, kind="ExternalInput")
    t_iota = nc.dram_tensor("iota", [P, P], bf16, kind="ExternalInput")
    t_w = [nc.dram_tensor("w1t", [IN, CA], bf16, kind="ExternalInput"),
           nc.dram_tensor("w2t", [HID, CA], bf16, kind="ExternalInput"),
           nc.dram_tensor("w3t", [HID, CA], bf16, kind="ExternalInput")]
    t_adf = [None,
             nc.dram_tensor("adf2", [HID, H], bf16, kind="ExternalInput"),
             nc.dram_tensor("adf3", [HID, H], bf16, kind="ExternalInput")]
    t_bias = nc.dram_tensor("bias", [P, 3, HID], f32, kind="ExternalInput")
    t_xres = nc.dram_tensor("xres", [2, NV, HID], f32, kind="ExternalInput")
    t_ad1 = nc.dram_tensor("ad1", [P, NCHUNK, 8], bf16, kind="ExternalInput")
    GMAX = 8
    t_out = nc.dram_tensor("out", [NV, HID], f32, kind="ExternalOutput")

    with tile.TileContext(nc) as tc:
        import contextlib
        with contextlib.ExitStack() as ctx:
            dram = ctx.enter_context(tc.tile_pool(name="dram", bufs=1, space="DRAM"))
            sb_res = ctx.enter_context(tc.tile_pool(name="res", bufs=1))
            sb_slab = ctx.enter_context(tc.tile_pool(name="slab", bufs=3))
            sb_p1 = ctx.enter_context(tc.tile_pool(name="p1", bufs=3))
            sb_s = ctx.enter_context(tc.tile_pool(name="s", bufs=2))
            sb_sm = ctx.enter_context(tc.tile_pool(name="sm", bufs=2))
            sb_ep = ctx.enter_context(tc.tile_pool(name="ep", bufs=2))
            ps_p1 = ctx.enter_context(tc.tile_pool(name="psp1", bufs=3, space="PSUM"))
            ps_acc = ctx.enter_context(tc.tile_pool(name="psacc", bufs=2, space="PSUM"))
            ps_ad = ctx.enter_context(tc.tile_pool(name="psad", bufs=1, space="PSUM"))
            ps_tr = ctx.enter_context(tc.tile_pool(name="pstr", bufs=1, space="PSUM"))

            nc.gpsimd.load_library(library_config.mlp)

            # DRAM intermediates (per-core private)
            T_lo = dram.tile([HALF + 128, ROW], bf16)
            T_hi = dram.tile([HALF + 128, ROW], bf16)
            hT_mine = [dram.tile([HID, NV], bf16, tag="hTm1", name="hTm1"),
                       dram.tile([HID, NV], bf16, tag="hTm2", name="hTm2")]
            hT_full = [dram.tile([NCORES, HID, NV], bf16, addr_space="Shared", tag="hTf1", name="hTf1"),
                       dram.tile([NCORES, HID, NV], bf16, addr_space="Shared", tag="hTf2", name="hTf2")]

            # ---- resident tiles ----
            gidx = sb_res.tile([P, TOT * 8], i16)
            oh = sb_res.tile([P, TOT], bf16)
            iota = sb_res.tile([P, P], bf16)
            bias = sb_res.tile([P, 3, HID], f32)
            wt = [sb_res.tile([IN, CA], bf16, tag="w1", name="w1"),
                  sb_res.tile([HID, CA], bf16, tag="w2", name="w2"),
                  sb_res.tile([HID, CA], bf16, tag="w3", name="w3")]
            adf = [None,
                   sb_res.tile([HID, H], bf16, tag="adf2", name="adf2"),
                   sb_res.tile([HID, H], bf16, tag="adf3", name="adf3")]
            ident = sb_res.tile([P, P], bf16)
            # ad values, ping-pong by layer parity: [128, 2, 49, 8]
            ad_sb = sb_res.tile([P, 2, NCHUNK, 8], bf16, tag="adsb", name="adsb")
            # two manually ping-ponged per-edge feature tiles (memset once so
            # pad slots always hold finite data)
            gbuf = [sb_res.tile([P, NSUBMAX, ROW], bf16, tag="g0", name="g0"),
                    sb_res.tile([P, NSUBMAX, ROW], bf16, tag="g1", name="g1")]
            make_identity(nc, ident[:])
            nc.sync.dma_start(gidx[:], t_gidx[:])
            nc.sync.dma_start(oh[:], t_oh[:])
            nc.sync.dma_start(iota[:], t_iota[:])
            nc.sync.dma_start(bias[:], t_bias[:])
            nc.sync.dma_start(ad_sb[:, 0], t_ad1[:])
            for i in range(3):
                nc.sync.dma_start(wt[i][:], t_w[i][:])
            for i in (1, 2):
                nc.sync.dma_start(adf[i][:], t_adf[i][:])

            # =========================================================
            # per-layer
            # =========================================================
            for layer in range(3):
                kin = IN if layer == 0 else HID

                # ---- phase 1: full-table xh_aug = in @ WaugT ----
                # Full 1536-row slabs interleave nodes so partition p emits 12
                # consecutive table rows -> one DMA of 128 x 9.2KB descriptors
                # instead of per-row 768B writes.  Remainders use _p1_tile.
                SLABW = 12 * P
                for half, T_tab in ((0, T_lo), (1, T_hi)):
                    if layer == 0:
                        srcs = [(t_xT, half * HALF, HALF, 0)]
                    else:
                        hfull = hT_full[layer - 1]
                        srcs = [(hfull[half * 4 + bb], 0, NV, bb * NV)
                                for bb in range(4)]
                    for srct, scol, slen, rbase in srcs:
                        for s0 in range(0, slen, SLABW):
                            w = min(SLABW, slen - s0)
                            slab = sb_slab.tile([kin, SLABW], bf16, tag="slab")
                            if layer == 0:
                                nc.sync.dma_start(
                                    slab[:, :w], srct[:, scol + s0:scol + s0 + w])
                            else:
                                nc.sync.dma_start(
                                    slab[:, :w], srct[:, s0:s0 + w])
                            if w == SLABW:
                                osb = sb_p1.tile([P, 12, ROW], bf16, tag="oslab")
                                sv = slab.rearrange("i (q j) -> i q j", j=12)
                                for j in range(12):
                                    ps = ps_p1.tile([P, CA], f32, space="PSUM",
                                                    tag="p1ps")
                                    nc.tensor.matmul(out=ps[:], lhsT=sv[:, :, j],
                                                     rhs=wt[layer][:],
                                                     start=True, stop=True)
                                    # split PSUM evacuation across ACT and DVE
                                    if j % 2 == 0:
                                        nc.scalar.activation(osb[:, j, 0:CA],
                                                             ps[:], AF.Copy)
                                    else:
                                        nc.vector.tensor_copy(osb[:, j, 0:CA],
                                                              ps[:])
                                dst = T_tab[rbase + s0:rbase + s0 + SLABW, :]
                                nc.sync.dma_start(
                                    dst.rearrange("(p j) c -> p j c", p=P),
                                    osb[:])
                            else:
                                for o in range(0, w, P):
                                    m = min(P, w - o)
                                    _p1_tile(nc, tc, slab[:, o:o + m], wt[layer],
                                             T_tab, rbase + s0 + o, m, kin,
                                             sb_p1, ps_p1, mybir)

                # ---- phase 2: per dst-chunk edge processing ----
                for k in range(NCHUNK):
                    m = min(P, NV - k * P)
                    nlo, nhi, nsub = int(NLO[k]), int(NHI[k]), int(NSUB[k])
                    so = int(sub_off[k])

                    g = gbuf[k % 2]
                    for s0, s1, tab in ((0, nlo, T_lo), (nlo, nsub, T_hi)):
                        nn = (s1 - s0) * P
                        nc.gpsimd.dma_gather(
                            g[:, s0:s1, :], tab[:],
                            gidx[:, (so + s0) * 8:(so + s1) * 8],
                            nn, nn, ROW,
                            single_packet=(nn <= 1024))

                    # S^T slice for this chunk
                    stt = sb_s.tile([P, NSUBMAX, P], bf16, tag="stt")
                    nc.sync.dma_start(stt[:, 0:nsub, :],
                                      t_stt[:, so * P:(so + nsub) * P])
                    # ad_dst per edge slot: adp[:, j, :] = S^T_j.T @ ad_chunk
                    adp = ps_ad.tile([P, NSUBMAX, H], f32, space="PSUM", tag="adp")
                    for j in range(nsub):
                        nc.tensor.matmul(out=adp[:, j, :], lhsT=stt[:, j, :],
                                         rhs=ad_sb[:, layer % 2, k, 0:H],
                                         start=True, stop=True)

                    # e = exp(prelu0.2(as + ad)), f32 logits
                    lgt = sb_sm.tile([P, NSUBMAX, H], f32, tag="lgt")
                    nc.vector.tensor_tensor(
                        out=lgt[:, 0:nsub, :], in0=g[:, 0:nsub, C:CA],
                        in1=adp[:, 0:nsub, :], op=ALU.add)
                    lk = sb_sm.tile([P, NSUBMAX, H], f32, tag="lk")
                    nc.vector.scalar_tensor_tensor(
                        out=lk[:, 0:nsub, :], in0=lgt[:, 0:nsub, :], scalar=0.2,
                        in1=lgt[:, 0:nsub, :], op0=ALU.mult, op1=ALU.max)
                    # e -> g as-columns (bf16, denominator rides the matmul)
                    nc.scalar.activation(g[:, 0:nsub, C:CA], lk[:, 0:nsub, :],
                                         AF.Exp)
                    # weight messages in place: g[:, :, 0:C] *= e (bcast over F)
                    nc.vector.tensor_tensor(
                        out=g[:, 0:nsub, 0:C].rearrange(
                            "p ns (h f) -> p ns h f", h=H),
                        in0=g[:, 0:nsub, 0:C].rearrange(
                            "p ns (h f) -> p ns h f", h=H),
                        in1=g[:, 0:nsub, C:CA].unsqueeze(-1).to_broadcast(
                            [P, nsub, H, F]),
                        op=ALU.mult)
                    # one-hot selection (slot-major)
                    S = sb_s.tile([P, NSUBMAX, P], bf16, tag="S")
                    nc.vector.tensor_tensor(
                        out=S[:, 0:nsub, :],
                        in0=oh[:, so:so + nsub].unsqueeze(-1).to_broadcast(
                            [P, nsub, P]),
                        in1=iota[:].unsqueeze(1).to_broadcast([P, nsub, P]),
                        op=ALU.is_equal)
                    # aggregate
                    acc = ps_acc.tile([P, CA], f32, space="PSUM", tag="acc")
                    for j in range(nsub):
                        nc.tensor.matmul(out=acc[:], lhsT=S[:, j, :],
                                         rhs=g[:, j, 0:CA],
                                         start=(j == 0), stop=(j == nsub - 1))

                    # ---- epilogue ----
                    rs = sb_ep.tile([P, H], f32, tag="rs")
                    nc.vector.reciprocal(rs[:], acc[:, C:CA])
                    o = sb_ep.tile([P, H, F], f32, tag="o")
                    nc.vector.tensor_tensor(
                        out=o[:],
                        in0=acc[:, 0:C].rearrange("p (h f) -> p h f", h=H),
                        in1=rs[:].unsqueeze(-1).to_broadcast([P, H, F]),
                        op=ALU.mult)
                    o2 = sb_ep.tile([P, 2, F], f32, tag="o2")
                    nc.vector.tensor_tensor(out=o2[:, 0, :], in0=o[:, 0, :],
                                            in1=o[:, 1, :], op=ALU.add)
                    nc.vector.tensor_tensor(out=o2[:, 1, :], in0=o[:, 2, :],
                                            in1=o[:, 3, :], op=ALU.add)
                    hsum = sb_ep.tile([P, F], f32, tag="hsum")
                    nc.vector.tensor_tensor(out=hsum[:], in0=o2[:, 0, :],
                                            in1=o2[:, 1, :], op=ALU.add)
                    # mean + bias
                    hb = sb_ep.tile([P, F], f32, tag="hb")
                    nc.vector.scalar_tensor_tensor(
                        out=hb[:], in0=hsum[:], scalar=0.25, op0=ALU.mult,
                        in1=bias[:, layer, :], op1=ALU.add)
                    # + residual (+ leaky for layers 0,1)
                    res = sb_ep.tile([P, F], f32, tag="res")
                    ri = 0 if layer < 2 else 1
                    nc.sync.dma_start(res[:m], t_xres[ri, k * P:k * P + m, :])
                    hf = sb_ep.tile([P, F], f32, tag="hf")
                    nc.vector.tensor_tensor(out=hf[:m], in0=hb[:m],
                                            in1=res[:m], op=ALU.add)
                    if layer < 2:
                        ho = sb_ep.tile([P, F], bf16, tag="ho")
                        if m < P:
                            nc.vector.memset(ho[:], 0)
                        nc.vector.scalar_tensor_tensor(
                            out=ho[:m], in0=hf[:m], scalar=0.01,
                            in1=hf[:m], op0=ALU.mult, op1=ALU.max)
                        # transpose (bf16) for next layer + store
                        trp = ps_tr.tile([HID, P], bf16, space="PSUM", tag="trp")
                        nc.tensor.transpose(out=trp[:], in_=ho[:, 0:HID],
                                            identity=ident[:])
                        hTt = sb_ep.tile([HID, P], bf16, tag="hTt")
                        nc.scalar.activation(hTt[:], trp[:], AF.Copy)
                        nc.sync.dma_start(hT_mine[layer][:, k * P:k * P + m],
                                          hTt[:, 0:m])
                        # next-layer ad for my rows -> SBUF (other parity)
                        adn = ps_tr.tile([P, H], f32, space="PSUM", tag="adn")
                        nc.tensor.matmul(out=adn[:], lhsT=hTt[:],
                                         rhs=adf[layer + 1][:],
                                         start=True, stop=True)
                        nc.scalar.activation(ad_sb[:, (layer + 1) % 2, k, 0:H],
                                             adn[:], AF.Copy)
                    else:
                        nc.sync.dma_start(t_out[k * P:k * P + m, :], hf[:m])

                # ---- allgather h^T ----
                if layer < 2:
                    nc.gpsimd.collective_compute(
                        "AllGather", mybir.AluOpType.bypass,
                        replica_groups=[list(range(NCORES))],
                        ins=[hT_mine[layer].opt()],
                        outs=[hT_full[layer].opt()])

    nc.compile()
    return nc


def _p1_tile(nc, tc, lhsT, wt, T_tab, rowbase, m, kin, sb_p1, ps_p1, mybir):
    """One phase-1 tile: rows [rowbase, rowbase+m) of the table."""
    f32 = mybir.dt.float32
    bf16 = mybir.dt.bfloat16
    AF = mybir.ActivationFunctionType
    ps = ps_p1.tile([P, CA], f32, space="PSUM", tag="p1ps")
    nc.tensor.matmul(out=ps[:m if m < P else P, :], lhsT=lhsT[:, 0:m],
                     rhs=wt[:], start=True, stop=True)
    sb = sb_p1.tile([P, ROW], bf16, tag="p1sb")
    nc.scalar.activation(sb[:m, 0:CA], ps[:m, :], AF.Copy)
    nc.sync.dma_start(T_tab[rowbase:rowbase + m, 0:CA], sb[:m, 0:CA])


def kernel(**inputs):
    from concourse.bass_utils import run_bass_kernel_spmd

    x = np.asarray(inputs["x"], np.float32)
    ei = np.asarray(inputs["edge_index"])
    weights = tuple(inputs[k] for k in
                    ("W1", "as1", "ad1", "b1", "W2", "as2", "ad2", "b2",
                     "W3", "as3", "ad3", "b3", "rw1", "rb1", "rw2", "rb2"))
    in_maps, meta = _preprocess(x, ei, weights)

    key = ("prog", tuple(meta["NLO"]), tuple(meta["NHI"]))
    if key not in _cache:
        _cache[key] = _build_program(meta)
    nc = _cache[key]

    res = run_bass_kernel_spmd(nc, in_maps, core_ids=list(range(NCORES)))
    out = np.concatenate([res.results[c]["out"] for c in range(NCORES)], axis=0)
    return out.astype(np.float32)
